# revision 1
# baseline (speedup 1.0000x reference)
"""Self-contained Trainium2 Bass kernel: GRU(relu, reset_after) + BN + Dense.

kernel(**inputs) takes FULL unsharded fp32 inputs, shards batch over 8
NeuronCores, runs the Bass kernel via run_bass_kernel_spmd, returns the
FULL [2048, 1] fp32 output.
"""
import numpy as np

"""GRU Bass kernel builder for TRN2 (one NeuronCore program, SPMD over 8 cores).

Layout (per core):
  B=256 batch (2 chunks of Bc=128), T timesteps, F=32 in-features, H=256 hidden.
  Everything transposed: H on partitions, batch on free dim.

DRAM inputs (host-prepped, fp16 unless noted):
  xT   [T*32, 256]  x transposed+interleaved: row t*32+f, col b
  wr   [128, 1536]  rec_kernel lhsT tiles: col block (c*6+m)*128 = rec[c*128:., m*128:.]
  wi   [128, 768]   input kernel replicated at partition groups 0/32/64/96
  bz   [128, 1024]  bias rows at partitions {0,32,64,96}; col block j = b_all[j*128:(j+1)*128]
                    b_all = [b_z(256) | b_r(256) | b_xh(256) | b_rh(256)]
  sv   [128, 2]     BN+dense folded scale s: col c = s[c*128:(c+1)*128]
  cv   [1, 1] f32   scalar constant folded from BN/dense biases
Output:
  y    [1, 256] f32  per-core output slice (before host concat)
"""
from contextlib import ExitStack

import concourse.bass as bass
import concourse.tile as tile
from concourse import bacc, mybir

F16 = mybir.dt.float16
F32 = mybir.dt.float32
AF = mybir.ActivationFunctionType


def build_gru_nc(T=256, debug=False):
    nc = bacc.Bacc("TRN2", num_devices=8, debug=debug)
    xT_d = nc.dram_tensor("xT", [T * 32, 256], F16, kind="ExternalInput")
    wr_d = nc.dram_tensor("wr", [128, 1536], F16, kind="ExternalInput")
    wi_d = nc.dram_tensor("wi", [128, 768], F16, kind="ExternalInput")
    bz_d = nc.dram_tensor("bz", [128, 1024], F16, kind="ExternalInput")
    sv_d = nc.dram_tensor("sv", [128, 2], F16, kind="ExternalInput")
    cv_d = nc.dram_tensor("cv", [1, 1], F32, kind="ExternalInput")
    brh_d = nc.dram_tensor("brh", [128, 2], F32, kind="ExternalInput")
    bxh_d = nc.dram_tensor("bxh", [128, 2], F32, kind="ExternalInput")
    y_d = nc.dram_tensor("y", [1, 256], F32, kind="ExternalOutput")

    with tile.TileContext(nc) as tc, ExitStack() as ctx:
        const = ctx.enter_context(tc.tile_pool(name="const", bufs=1))
        hpool = [
            ctx.enter_context(tc.tile_pool(name=f"h{c}", bufs=2)) for c in (0, 1)
        ]
        gpool = [
            ctx.enter_context(tc.tile_pool(name=f"g{c}", bufs=2)) for c in (0, 1)
        ]
        zrpool = [
            ctx.enter_context(
                tc.tile_pool(name=f"zr{c}", bufs=2, space=bass.MemorySpace.PSUM)
            )
            for c in (0, 1)
        ]
        xrpool = [
            ctx.enter_context(
                tc.tile_pool(name=f"xr{c}", bufs=2, space=bass.MemorySpace.PSUM)
            )
            for c in (0, 1)
        ]

        # ---- constants / weights ----
        xsb = const.tile([128, (T * 32 // 128) * 256], F16)  # x^T tiles, col blk j
        wr = const.tile([128, 1536], F16)
        wi = const.tile([128, 768], F16)
        bz = const.tile([128, 1024], F16)
        sv = const.tile([128, 2], F16)
        cv = const.tile([1, 1], F32)
        brh = const.tile([128, 2], F32)
        bxh = const.tile([128, 2], F32)
        ones = const.tile([128, 128], F16)

        nc.sync.dma_start(wr[:], wr_d.ap())
        nc.sync.dma_start(wi[:], wi_d.ap())
        nc.sync.dma_start(bz[:], bz_d.ap())
        nc.sync.dma_start(sv[:], sv_d.ap())
        nc.sync.dma_start(cv[:], cv_d.ap())
        nc.sync.dma_start(brh[:], brh_d.ap())
        nc.sync.dma_start(bxh[:], bxh_d.ap())
        nc.vector.memset(ones[:], 1.0)

        # x load: split into up to 4 chunks of row-tiles for pipelining
        ntile = T * 32 // 128  # 64 for T=256
        nchunk = min(4, ntile)
        per = ntile // nchunk
        for jc in range(nchunk):
            src = xT_d.ap()[jc * per * 128 : (jc + 1) * per * 128, :]
            src = src.rearrange("(j p) b -> p j b", p=128)
            dst = xsb[:, jc * per * 256 : (jc + 1) * per * 256]
            dst = dst.rearrange("p (j b) -> p j b", b=256)
            nc.sync.dma_start(dst, src)

        # ---- initial hidden state ----
        h = []
        for c in (0, 1):
            h0 = hpool[c].tile([128, 256], F16)
            nc.vector.memset(h0[:], 0.0)
            h.append(h0)

        def x_phase(t, c):
            """Emit x-projection + bias matmuls for step t, chunk c."""
            zr = zrpool[c].tile([128, 512], F32)
            xr = xrpool[c].tile([128, 512], F32)
            g = t % 4
            col0 = (t // 4) * 256 + c * 128
            xrhs = xsb[32 * g : 32 * g + 32, col0 : col0 + 128]

            def xmm(m, out, start):
                lhsT = wi[32 * g : 32 * g + 32, m * 128 : (m + 1) * 128]
                nc.tensor.matmul(
                    out, lhsT, xrhs, start=start, stop=False,
                    tile_position=(32 * g, 0),
                )

            def bmm(j, out):
                # same row group as the x-MMs: serializes in-array (no
                # cross-group write race into a shared PSUM region)
                lhsT = bz[32 * g : 32 * g + 1, j * 128 : (j + 1) * 128]
                rhs = ones[32 * g : 32 * g + 1, 0:128]
                nc.tensor.matmul(
                    out, lhsT, rhs, start=False, stop=False,
                    tile_position=(32 * g, 0),
                )

            # zr bank: z0 z1 r0 r1 ; xr bank: xh0 xh1 | rh0 rh1
            xmm(0, zr[:, 0:128], True)       # z0 clears zr bank
            bmm(0, zr[:, 0:128])
            xmm(1, zr[:, 128:256], False)
            bmm(1, zr[:, 128:256])
            xmm(2, zr[:, 256:384], False)
            bmm(2, zr[:, 256:384])
            xmm(3, zr[:, 384:512], False)
            bmm(3, zr[:, 384:512])
            xmm(4, xr[:, 0:128], True)       # xh0 clears xr bank
            xmm(5, xr[:, 128:256], False)
            return zr, xr

        cur = [x_phase(0, 0), x_phase(0, 1)]

        for t in range(T):
            for c in (0, 1):
                zr, xr = cur[c]
                hc = h[c]
                # rec matmuls: r (m=2,3) first, then rh (m=4,5), then z (m=0,1)
                for m in (2, 3, 4, 5, 0, 1):
                    if m < 4:
                        out = zr[:, m * 128 : (m + 1) * 128]
                    else:
                        out = xr[:, 256 + (m - 4) * 128 : 256 + (m - 3) * 128]
                    for ct in (0, 1):
                        lhsT = wr[:, (ct * 6 + m) * 128 : (ct * 6 + m + 1) * 128]
                        rhs = hc[:, ct * 128 : (ct + 1) * 128]
                        stop = (ct == 1) and (m in (1, 5))
                        nc.tensor.matmul(out, lhsT, rhs, start=False, stop=stop)

                r_sb = gpool[c].tile([128, 256], F16, tag="r")
                z_sb = gpool[c].tile([128, 256], F16, tag="z")
                xh_sb = gpool[c].tile([128, 256], F16, tag="xh")
                nc.scalar.activation(r_sb[:], zr[:, 256:512], AF.Sigmoid)
                nc.scalar.activation(xh_sb[:], xr[:, 0:256], AF.Copy)
                nc.scalar.activation(z_sb[:], zr[:, 0:256], AF.Sigmoid)

                p = gpool[c].tile([128, 256], F16, tag="p")
                for ct in (0, 1):
                    nc.vector.scalar_tensor_tensor(
                        p[:, ct * 128 : (ct + 1) * 128],
                        xr[:, 256 + ct * 128 : 256 + (ct + 1) * 128],
                        brh[:, ct : ct + 1],
                        r_sb[:, ct * 128 : (ct + 1) * 128],
                        op0=mybir.AluOpType.add,
                        op1=mybir.AluOpType.mult,
                    )
                pre = gpool[c].tile([128, 256], F16, tag="pre")
                nc.vector.tensor_add(pre[:], xh_sb[:], p[:])
                hh = gpool[c].tile([128, 256], F16, tag="hh")
                for ct in (0, 1):
                    nc.vector.tensor_scalar(
                        hh[:, ct * 128 : (ct + 1) * 128],
                        pre[:, ct * 128 : (ct + 1) * 128],
                        bxh[:, ct : ct + 1],
                        0.0,
                        op0=mybir.AluOpType.add,
                        op1=mybir.AluOpType.max,
                    )
                d = gpool[c].tile([128, 256], F16, tag="d")
                nc.vector.tensor_sub(d[:], hc[:], hh[:])
                e = gpool[c].tile([128, 256], F16, tag="e")
                nc.vector.tensor_mul(e[:], z_sb[:], d[:])
                hn = hpool[c].tile([128, 256], F16)
                nc.vector.tensor_add(hn[:], hh[:], e[:])
                h[c] = hn

                if t + 1 < T:
                    cur[c] = x_phase(t + 1, c)

        # ---- BN + dense epilogue: y = s . h + c ----
        fin = zrpool[0].tile([128, 512], F32, tag="zr")
        first = True
        for c in (0, 1):
            for ct in (0, 1):
                nc.tensor.matmul(
                    fin[0:1, c * 128 : (c + 1) * 128],
                    sv[:, ct : ct + 1],
                    h[c][:, ct * 128 : (ct + 1) * 128],
                    start=first,
                    stop=(c == 1 and ct == 1),
                )
                first = False
        ysb = const.tile([1, 256], F32)
        nc.vector.tensor_scalar_add(ysb[:], fin[0:1, 0:256], cv[0:1, 0:1])
        nc.sync.dma_start(y_d.ap(), ysb[:])

    nc.compile()
    return nc


BN_EPS = 1e-3


def prep_core_inputs(x_core, kernel, rec_kernel, bias, gamma, beta,
                     moving_mean, moving_var, dense_w, dense_b):
    """Host-side prep of one core's input dict. x_core: [B=256, T, 32] f32."""
    import numpy as np
    B, T, F = x_core.shape
    H = 256
    xT = np.ascontiguousarray(
        x_core.astype(np.float16).transpose(1, 2, 0).reshape(T * F, B)
    )
    rec = rec_kernel.astype(np.float16)
    wr = np.concatenate(
        [rec[ct * 128 : (ct + 1) * 128, m * 128 : (m + 1) * 128]
         for ct in (0, 1) for m in range(6)],
        axis=1,
    )
    wi = np.tile(kernel.astype(np.float16), (4, 1))
    b_z = bias[0, 0:256] + bias[1, 0:256]
    b_r = bias[0, 256:512] + bias[1, 256:512]
    b_xh = bias[0, 512:768]
    b_rh = bias[1, 512:768]
    b_all = np.concatenate([b_z, b_r]).astype(np.float16)
    bz = np.zeros((128, 1024), np.float16)
    for g in (0, 32, 64, 96):
        bz[g, 0:512] = b_all
    brh = np.stack([b_rh[:128], b_rh[128:]], axis=1).astype(np.float32)
    bxh = np.stack([b_xh[:128], b_xh[128:]], axis=1).astype(np.float32)
    rs = 1.0 / np.sqrt(moving_var + BN_EPS)
    s = (gamma * rs * dense_w[:, 0]).astype(np.float16)
    sv = np.stack([s[:128], s[128:]], axis=1)
    cc = dense_b[0] + np.sum((beta - moving_mean * gamma * rs) * dense_w[:, 0])
    cv = np.array([[cc]], np.float32)
    return {
        "xT": np.ascontiguousarray(xT),
        "wr": np.ascontiguousarray(wr),
        "wi": np.ascontiguousarray(wi),
        "bz": bz,
        "sv": np.ascontiguousarray(sv),
        "cv": cv,
        "brh": np.ascontiguousarray(brh),
        "bxh": np.ascontiguousarray(bxh),
    }


_NC_CACHE = {}


def _get_nc():
    if "nc" not in _NC_CACHE:
        _NC_CACHE["nc"] = build_gru_nc(T=256)
    return _NC_CACHE["nc"]


def kernel(x, kernel, rec_kernel, bias, gamma, beta, moving_mean, moving_var,
           dense_w, dense_b):
    from concourse.bass_utils import run_bass_kernel_spmd

    x = np.asarray(x, dtype=np.float32)
    args = [np.asarray(a, dtype=np.float32) for a in
            (kernel, rec_kernel, bias, gamma, beta, moving_mean, moving_var,
             dense_w, dense_b)]
    nc = _get_nc()
    n_cores = 8
    nb = x.shape[0] // n_cores
    in_maps = [prep_core_inputs(x[i * nb : (i + 1) * nb], *args)
               for i in range(n_cores)]
    res = run_bass_kernel_spmd(nc, in_maps, core_ids=list(range(n_cores)))
    return np.concatenate(
        [res.results[i]["y"].reshape(nb, 1) for i in range(n_cores)], axis=0
    ).astype(np.float32)



# revision 5
# speedup vs baseline: 4.6332x; 4.6332x over previous
"""GRU Bass kernel v2 for TRN2 — chain-shortened, bias-folded design.

Per core: B=256 batch (2 chunks of 128), T=256 steps, F=32, H=256.
Transposed layout: hidden/gate dims on partitions (2 col-blocks of 128),
batch on free dim.

PSUM discipline: one accumulation group per 2KB bank (start=True lazily
zeros the whole bank; no reads until the bank's group stops). Per chunk:
  R  bank [128,256] f32: [r0_t|r1_t]   x-MMs + rec-r, closes after 4 rec MMs
  Z  bank [128,256] f32: [z0_t|z1_t]   x-MMs + rec-z
  RH bank [128,256] f32: [rh0_t|rh1_t] bias-MMs + rec-rh
  XH bank [128,512] f32: [xh0_t xh0_u xh1_t xh1_u]  pair x-MMs only
All bufs=1 — rotation is WAR-ordered behind each bank's last reader.

x data (xp) pair layout: pair p = (t=2p, u=2p+1) at row parity g=p%2
(rows 64g..64g+33: 32 features + ones row), cols (p//2)*512 + c*256 +
tu*128. The ones row folds biases b_z+b_rz, b_r+b_rr, b_xh via wi's
33rd row; b_rh comes from K=1 bias MMs.

Per step chain: rec r(4) -> sig-r ; rec rh(4) ; rec z(4) -> sig-z
  p   = rh_psum * r_sb          (tt, PSUM 1x)
  pre = p + xh_sb               (tt, fp16 2x; xh copied by ACT pair-ahead)
  zc  = 1 - z                   (ts 4x)
  w   = relu(pre) * zc          (stt)
  u   = z * h                   (GPSIMD, off-chain)
  hn  = u + w                   (tt)
"""
from contextlib import ExitStack

import concourse.bass as bass
import concourse.tile as tile
from concourse import bacc, mybir

F16 = mybir.dt.float16
F32 = mybir.dt.float32
AF = mybir.ActivationFunctionType
ALU = mybir.AluOpType


def build_gru_nc(T=256, debug=False):
    assert T % 2 == 0
    npair = T // 2
    nc = bacc.Bacc("TRN2", num_devices=8, debug=debug)
    xp_d = nc.dram_tensor("xp", [128, (npair + 1) // 2 * 512], F16,
                          kind="ExternalInput")
    wr_d = nc.dram_tensor("wr", [128, 1536], F16, kind="ExternalInput")
    wi_d = nc.dram_tensor("wi", [128, 768], F16, kind="ExternalInput")
    brh_d = nc.dram_tensor("brh", [1, 256], F16, kind="ExternalInput")
    sv_d = nc.dram_tensor("sv", [128, 2], F16, kind="ExternalInput")
    cv_d = nc.dram_tensor("cv", [1, 1], F32, kind="ExternalInput")
    y_d = nc.dram_tensor("y", [1, 256], F32, kind="ExternalOutput")

    XCOLS = (npair + 1) // 2 * 512

    with tile.TileContext(nc) as tc, ExitStack() as ctx:
        const = ctx.enter_context(tc.tile_pool(name="const", bufs=1))
        hpool = [ctx.enter_context(tc.tile_pool(name=f"h{c}", bufs=4))
                 for c in (0, 1)]
        gpool = [ctx.enter_context(tc.tile_pool(name=f"g{c}", bufs=4))
                 for c in (0, 1)]
        xhpool = [ctx.enter_context(tc.tile_pool(name=f"xh{c}", bufs=2))
                  for c in (0, 1)]
        PS = bass.MemorySpace.PSUM
        rpool = [ctx.enter_context(tc.tile_pool(name=f"rp{c}", bufs=1, space=PS))
                 for c in (0, 1)]
        zpool = [ctx.enter_context(tc.tile_pool(name=f"zp{c}", bufs=1, space=PS))
                 for c in (0, 1)]
        rhpool = [ctx.enter_context(tc.tile_pool(name=f"rhp{c}", bufs=1, space=PS))
                  for c in (0, 1)]
        xhps = [ctx.enter_context(tc.tile_pool(name=f"xhp{c}", bufs=1, space=PS))
                for c in (0, 1)]

        # ---- constants / weights ----
        xsb = const.tile([128, XCOLS], F16)
        wr = const.tile([128, 1536], F16)
        wi = const.tile([128, 768], F16)
        brh = const.tile([1, 256], F16)
        sv = const.tile([128, 2], F16)
        cv = const.tile([1, 1], F32)
        ones = const.tile([1, 256], F16)

        nc.sync.dma_start(wr[:], wr_d.ap())
        nc.sync.dma_start(wi[:], wi_d.ap())
        nc.sync.dma_start(brh[:], brh_d.ap())
        nc.sync.dma_start(sv[:], sv_d.ap())
        nc.sync.dma_start(cv[:], cv_d.ap())
        nc.vector.memset(ones[:], 1.0)

        nchunk = 8
        per = XCOLS // nchunk
        for jc in range(nchunk):
            nc.sync.dma_start(
                xsb[:, jc * per : (jc + 1) * per],
                xp_d.ap()[:, jc * per : (jc + 1) * per],
            )

        # ---- initial hidden state (h0 of chunk 1 skewed for anti-phase) ----
        h = [None, None]
        h0a = hpool[0].tile([128, 256], F16)
        nc.vector.memset(h0a[:], 0.0)
        h[0] = h0a
        skew_src = [None]

        def xcol(t, c):
            p = t // 2
            return (p // 2) * 512 + c * 256 + (t % 2) * 128

        def xg(t):
            return ((t // 2) % 2) * 64

        def x_step(t, c):
            """Per-step x-MMs: r/z projections + rh bias into fresh banks."""
            g = xg(t)
            rt = rpool[c].tile([128, 256], F32, tag="r")
            zt = zpool[c].tile([128, 256], F32, tag="z")
            rht = rhpool[c].tile([128, 256], F32, tag="rh")
            col = xcol(t, c)
            xrhs = xsb[g : g + 64, col : col + 128]
            # wi gate blocks: m=0,1 z ; m=2,3 r ; m=4,5 xh
            for blk in (0, 1):
                nc.tensor.matmul(
                    rt[:, blk * 128 : (blk + 1) * 128],
                    wi[g : g + 64, (2 + blk) * 128 : (3 + blk) * 128],
                    xrhs, start=(blk == 0), stop=False,
                    tile_position=(g, 0))
                nc.tensor.matmul(
                    zt[:, blk * 128 : (blk + 1) * 128],
                    wi[g : g + 64, blk * 128 : (blk + 1) * 128],
                    xrhs, start=(blk == 0), stop=False,
                    tile_position=(g, 0))
                nc.tensor.matmul(
                    rht[:, blk * 128 : (blk + 1) * 128],
                    brh[0:1, blk * 128 : (blk + 1) * 128],
                    ones[0:1, 0:128],
                    start=(blk == 0), stop=False)
            return rt, zt, rht

        def x_pair(p, c):
            """Pair xh MMs (N=256 covering both steps); copies emitted
            separately per step-half to keep long ACT ops off the sig path."""
            xhp = xhps[c].tile([128, 512], F32, tag="xhp")
            g = (p % 2) * 64
            colbase = (p // 2) * 512 + c * 256
            xrhs = xsb[g : g + 64, colbase : colbase + 256]
            for blk in (0, 1):
                nc.tensor.matmul(
                    xhp[:, blk * 256 : (blk + 1) * 256],
                    wi[g : g + 64, (4 + blk) * 128 : (5 + blk) * 128],
                    xrhs, start=(blk == 0), stop=(blk == 1),
                    tile_position=(g, 0))
            xh_sb = xhpool[c].tile([128, 512], F16)
            return xhp, xh_sb

        def xh_copy_half(xhp, xh_sb, tu):
            src = xhp[:].rearrange("p (k b) -> p k b", k=2, b=256)
            dst = xh_sb[:].rearrange("p (k b) -> p k b", k=2, b=256)
            nc.scalar.activation(dst[:, :, tu * 128 : tu * 128 + 128],
                                 src[:, :, tu * 128 : tu * 128 + 128],
                                 AF.Copy)

        def step(t, c, rt, zt, rht, xh_sb):
            tu = t % 2
            hc = h[c]

            def rec_mm1(m, ct, out, stop):
                lhsT = wr[:, (ct * 6 + m) * 128 : (ct * 6 + m + 1) * 128]
                rhs = hc[:, ct * 128 : (ct + 1) * 128]
                nc.tensor.matmul(out, lhsT, rhs, start=False, stop=stop)

            def rec_mm(m, out, stop):
                rec_mm1(m, 0, out, False)
                rec_mm1(m, 1, out, stop)

            r_sb = gpool[c].tile([128, 256], F16, tag="r")
            z_sb = gpool[c].tile([128, 256], F16, tag="z")

            # rec r (closes R bank -> sig-r fires after 4 MMs)
            rec_mm(2, rt[:, 0:128], False)
            rec_mm(3, rt[:, 128:256], True)
            nc.scalar.activation(r_sb[:], rt[:, 0:256], AF.Sigmoid)
            # rec rh (closes RH bank)
            rec_mm(4, rht[:, 0:128], False)
            rec_mm(5, rht[:, 128:256], True)
            # rec z (closes Z bank), sig-z
            rec_mm(0, zt[:, 0:128], False)
            rec_mm(1, zt[:, 128:256], True)
            nc.scalar.activation(z_sb[:], zt[:, 0:256], AF.Sigmoid)

            xh_in = xh_sb[:].rearrange("p (k b) -> p k b", k=2, b=256)
            xh_in = xh_in[:, :, tu * 128 : tu * 128 + 128]

            # off-chain op on GPSIMD: u = z * h
            zc = gpool[c].tile([128, 256], F16, tag="zc")
            u_t = gpool[c].tile([128, 256], F16, tag="u")
            nc.gpsimd.tensor_tensor(u_t[:], z_sb[:], hc[:], ALU.mult)

            p_t = gpool[c].tile([128, 256], F16, tag="p")
            pre = gpool[c].tile([128, 256], F16, tag="pre")
            w_t = gpool[c].tile([128, 256], F16, tag="w")
            nc.vector.tensor_tensor(p_t[:], rht[:], r_sb[:], ALU.mult)
            pre_out = pre[:].rearrange("p (k b) -> p k b", k=2, b=128)
            p_in = p_t[:].rearrange("p (k b) -> p k b", k=2, b=128)
            nc.vector.tensor_tensor(pre_out, p_in, xh_in, ALU.add)
            # zc emitted late so its sig-z dep never clogs the queues
            nc.vector.tensor_scalar(zc[:], z_sb[:], -1.0, 1.0,
                                    op0=ALU.mult, op1=ALU.add)
            # w = relu(pre) * (1-z)
            nc.vector.scalar_tensor_tensor(
                w_t[:], pre[:], 0.0, zc[:], op0=ALU.max, op1=ALU.mult)
            if skew_src[0] is None:
                skew_src[0] = p_t
            hn = hpool[c].tile([128, 256], F16)
            nc.vector.tensor_tensor(hn[:], u_t[:], w_t[:], ALU.add)
            h[c] = hn

        # priming: x tiles for t=0, xh pair 0 + its first-half copy
        cur = [x_step(0, 0), x_step(0, 1)]
        xpair_cur = [x_pair(0, 0), x_pair(0, 1)]
        for c in (0, 1):
            xh_copy_half(*xpair_cur[c], 0)
        for t in range(T):
            for c in (0, 1):
                if t == 0 and c == 1:
                    h0b = hpool[1].tile([128, 256], F16)
                    nc.vector.memset(h0b[:], 0.0)
                    h[1] = h0b
                rt, zt, rht = cur[c]
                step(t, c, rt, zt, rht, xpair_cur[c][1])
                if t + 1 < T:
                    cur[c] = x_step(t + 1, c)
            if t % 2 == 0 and t + 1 < T:
                # second-half copy of the current pair (needed at t+1)
                for c in (0, 1):
                    xh_copy_half(*xpair_cur[c], 1)
            elif t % 2 == 1 and t + 2 < T:
                # next pair's MMs + first-half copy (needed at t+2)
                for c in (0, 1):
                    xpair_cur[c] = x_pair((t + 1) // 2, c)
                    xh_copy_half(*xpair_cur[c], 0)

        # ---- BN + dense epilogue: y = s . h + c ----
        fin = zpool[0].tile([128, 256], F32, tag="z")
        first = True
        for c in (0, 1):
            for ct in (0, 1):
                nc.tensor.matmul(
                    fin[0:1, c * 128 : (c + 1) * 128],
                    sv[:, ct : ct + 1],
                    h[c][:, ct * 128 : (ct + 1) * 128],
                    start=first,
                    stop=(c == 1 and ct == 1),
                )
                first = False
        ysb = const.tile([1, 256], F32)
        nc.vector.tensor_scalar_add(ysb[:], fin[0:1, 0:256], cv[0:1, 0:1])
        nc.sync.dma_start(y_d.ap(), ysb[:])

    nc.compile()
    return nc


BN_EPS = 1e-3


def prep_core_inputs(x_core, kernel, rec_kernel, bias, gamma, beta,
                     moving_mean, moving_var, dense_w, dense_b):
    """Host-side prep of one core's input dict. x_core: [B=256, T, 32] f32."""
    import numpy as np
    B, T, F = x_core.shape
    npair = T // 2
    xp = np.zeros((128, (npair + 1) // 2 * 512), np.float16)
    xt = np.ascontiguousarray(x_core.astype(np.float16).transpose(1, 2, 0))  # [T,F,B]
    for p in range(npair):
        g, colbase = p % 2, (p // 2) * 512
        for c in (0, 1):
            for tu in (0, 1):
                blk = xt[2 * p + tu][:, c * 128 : (c + 1) * 128]  # [F,128]
                cols = slice(colbase + c * 256 + tu * 128,
                             colbase + c * 256 + tu * 128 + 128)
                xp[64 * g : 64 * g + 32, cols] = blk
                xp[64 * g + 32, cols] = 1.0

    rec = rec_kernel.astype(np.float16)
    wr = np.concatenate(
        [rec[ct * 128 : (ct + 1) * 128, m * 128 : (m + 1) * 128]
         for ct in (0, 1) for m in range(6)],
        axis=1,
    )
    b_comb = np.concatenate([
        bias[0, 0:256] + bias[1, 0:256],
        bias[0, 256:512] + bias[1, 256:512],
        bias[0, 512:768],
    ])
    wi = np.zeros((128, 768), np.float16)
    for g in (0, 1):
        wi[64 * g : 64 * g + 32, :] = kernel.astype(np.float16)
        wi[64 * g + 32, :] = b_comb.astype(np.float16)
    brh = bias[1, 512:768].astype(np.float16).reshape(1, 256)

    rs = 1.0 / np.sqrt(moving_var + BN_EPS)
    s = (gamma * rs * dense_w[:, 0]).astype(np.float16)
    sv = np.stack([s[:128], s[128:]], axis=1)
    cc = dense_b[0] + np.sum((beta - moving_mean * gamma * rs) * dense_w[:, 0])
    cv = np.array([[cc]], np.float32)
    return {
        "xp": np.ascontiguousarray(xp),
        "wr": np.ascontiguousarray(wr),
        "wi": wi,
        "brh": brh,
        "sv": np.ascontiguousarray(sv),
        "cv": cv,
    }

_NC_CACHE = {}


def _get_nc():
    if "nc" not in _NC_CACHE:
        _NC_CACHE["nc"] = build_gru_nc(T=256)
    return _NC_CACHE["nc"]


def kernel(x, kernel, rec_kernel, bias, gamma, beta, moving_mean, moving_var,
           dense_w, dense_b):
    import numpy as np
    from concourse.bass_utils import run_bass_kernel_spmd

    x = np.asarray(x, dtype=np.float32)
    args = [np.asarray(a, dtype=np.float32) for a in
            (kernel, rec_kernel, bias, gamma, beta, moving_mean, moving_var,
             dense_w, dense_b)]
    nc = _get_nc()
    n_cores = 8
    nb = x.shape[0] // n_cores
    in_maps = [prep_core_inputs(x[i * nb : (i + 1) * nb], *args)
               for i in range(n_cores)]
    res = run_bass_kernel_spmd(nc, in_maps, core_ids=list(range(n_cores)))
    return np.concatenate(
        [res.results[i]["y"].reshape(nb, 1) for i in range(n_cores)], axis=0
    ).astype(np.float32)


# revision 7
# speedup vs baseline: 4.6489x; 1.0034x over previous
"""GRU Bass kernel v2 for TRN2 — chain-shortened, bias-folded design.

Per core: B=256 batch (2 chunks of 128), T=256 steps, F=32, H=256.
Transposed layout: hidden/gate dims on partitions (2 col-blocks of 128),
batch on free dim.

PSUM discipline: one accumulation group per 2KB bank (start=True lazily
zeros the whole bank; no reads until the bank's group stops). Per chunk:
  R  bank [128,256] f32: [r0_t|r1_t]   x-MMs + rec-r, closes after 4 rec MMs
  Z  bank [128,256] f32: [z0_t|z1_t]   x-MMs + rec-z
  RH bank [128,256] f32: [rh0_t|rh1_t] bias-MMs + rec-rh
  XH bank [128,512] f32: [xh0_t xh0_u xh1_t xh1_u]  pair x-MMs only
All bufs=1 — rotation is WAR-ordered behind each bank's last reader.

x data (xp) pair layout: pair p = (t=2p, u=2p+1) at row parity g=p%2
(rows 64g..64g+33: 32 features + ones row), cols (p//2)*512 + c*256 +
tu*128. The ones row folds biases b_z+b_rz, b_r+b_rr, b_xh via wi's
33rd row; b_rh comes from K=1 bias MMs.

Per step chain: rec r(4) -> sig-r ; rec rh(4) ; rec z(4) -> sig-z
  p   = rh_psum * r_sb          (tt, PSUM 1x)
  pre = p + xh_sb               (tt, fp16 2x; xh copied by ACT pair-ahead)
  zc  = 1 - z                   (ts 4x)
  w   = relu(pre) * zc          (stt)
  u   = z * h                   (GPSIMD, off-chain)
  hn  = u + w                   (tt)
"""
from contextlib import ExitStack

import concourse.bass as bass
import concourse.tile as tile
from concourse import bacc, mybir

F16 = mybir.dt.float16
F32 = mybir.dt.float32
AF = mybir.ActivationFunctionType
ALU = mybir.AluOpType


def build_gru_nc(T=256, debug=False):
    assert T % 2 == 0
    npair = T // 2
    nc = bacc.Bacc("TRN2", num_devices=8, debug=debug)
    xp_d = nc.dram_tensor("xp", [128, (npair + 1) // 2 * 512], F16,
                          kind="ExternalInput")
    wr_d = nc.dram_tensor("wr", [128, 1536], F16, kind="ExternalInput")
    wi_d = nc.dram_tensor("wi", [128, 768], F16, kind="ExternalInput")
    brh_d = nc.dram_tensor("brh", [1, 256], F16, kind="ExternalInput")
    sv_d = nc.dram_tensor("sv", [128, 2], F16, kind="ExternalInput")
    cv_d = nc.dram_tensor("cv", [1, 1], F32, kind="ExternalInput")
    y_d = nc.dram_tensor("y", [1, 256], F32, kind="ExternalOutput")

    XCOLS = (npair + 1) // 2 * 512

    with tile.TileContext(nc) as tc, ExitStack() as ctx:
        const = ctx.enter_context(tc.tile_pool(name="const", bufs=1))
        hpool = [ctx.enter_context(tc.tile_pool(name=f"h{c}", bufs=4))
                 for c in (0, 1)]
        gpool = [ctx.enter_context(tc.tile_pool(name=f"g{c}", bufs=4))
                 for c in (0, 1)]
        xhpool = [ctx.enter_context(tc.tile_pool(name=f"xh{c}", bufs=2))
                  for c in (0, 1)]
        PS = bass.MemorySpace.PSUM
        rpool = [ctx.enter_context(tc.tile_pool(name=f"rp{c}", bufs=1, space=PS))
                 for c in (0, 1)]
        zpool = [ctx.enter_context(tc.tile_pool(name=f"zp{c}", bufs=1, space=PS))
                 for c in (0, 1)]
        rhpool = [ctx.enter_context(tc.tile_pool(name=f"rhp{c}", bufs=1, space=PS))
                  for c in (0, 1)]
        xhps = [ctx.enter_context(tc.tile_pool(name=f"xhp{c}", bufs=1, space=PS))
                for c in (0, 1)]

        # ---- constants / weights ----
        xsb = const.tile([128, XCOLS], F16)
        wr = const.tile([128, 1536], F16)
        wi = const.tile([128, 768], F16)
        brh = const.tile([1, 256], F16)
        sv = const.tile([128, 2], F16)
        cv = const.tile([1, 1], F32)
        ones = const.tile([1, 256], F16)

        nc.sync.dma_start(wr[:], wr_d.ap())
        nc.sync.dma_start(wi[:], wi_d.ap())
        nc.sync.dma_start(brh[:], brh_d.ap())
        nc.sync.dma_start(sv[:], sv_d.ap())
        nc.sync.dma_start(cv[:], cv_d.ap())
        nc.vector.memset(ones[:], 1.0)
        # dummy sigmoid loads the sigmoid ACT table set (which also holds
        # Copy) up front, avoiding a 2.7us mid-stream table switch when the
        # first real sigmoid runs
        warm = const.tile([1, 1], F16)
        nc.scalar.activation(warm[:], ones[0:1, 0:1], AF.Sigmoid)

        # x load: small head chunk so the first pairs' x-MMs unblock early,
        # then 4 large chunks (fewer serial HWDGE descriptors)
        head = min(1024, XCOLS)
        nc.sync.dma_start(xsb[:, 0:head], xp_d.ap()[:, 0:head])
        rem = XCOLS - head
        if rem > 0:
            nch = 4
            per = max(rem // nch, 1)
            lo = head
            while lo < XCOLS:
                hi = min(lo + per, XCOLS)
                if XCOLS - hi < per:
                    hi = XCOLS
                nc.sync.dma_start(xsb[:, lo:hi], xp_d.ap()[:, lo:hi])
                lo = hi

        # ---- initial hidden state (h0 of chunk 1 skewed for anti-phase) ----
        h = [None, None]
        h0a = hpool[0].tile([128, 256], F16)
        nc.vector.memset(h0a[:], 0.0)
        h[0] = h0a
        skew_src = [None]

        def xcol(t, c):
            p = t // 2
            return (p // 2) * 512 + c * 256 + (t % 2) * 128

        def xg(t):
            return ((t // 2) % 2) * 64

        def x_step(t, c):
            """Per-step x-MMs: r/z projections + rh bias into fresh banks."""
            g = xg(t)
            rt = rpool[c].tile([128, 256], F32, tag="r")
            zt = zpool[c].tile([128, 256], F32, tag="z")
            rht = rhpool[c].tile([128, 256], F32, tag="rh")
            col = xcol(t, c)
            xrhs = xsb[g : g + 64, col : col + 128]
            # wi gate blocks: m=0,1 z ; m=2,3 r ; m=4,5 xh
            for blk in (0, 1):
                nc.tensor.matmul(
                    rt[:, blk * 128 : (blk + 1) * 128],
                    wi[g : g + 64, (2 + blk) * 128 : (3 + blk) * 128],
                    xrhs, start=(blk == 0), stop=False,
                    tile_position=(g, 0))
                nc.tensor.matmul(
                    zt[:, blk * 128 : (blk + 1) * 128],
                    wi[g : g + 64, blk * 128 : (blk + 1) * 128],
                    xrhs, start=(blk == 0), stop=False,
                    tile_position=(g, 0))
                nc.tensor.matmul(
                    rht[:, blk * 128 : (blk + 1) * 128],
                    brh[0:1, blk * 128 : (blk + 1) * 128],
                    ones[0:1, 0:128],
                    start=(blk == 0), stop=False)
            return rt, zt, rht

        def x_pair(p, c):
            """Pair xh MMs (N=256 covering both steps); copies emitted
            separately per step-half to keep long ACT ops off the sig path."""
            xhp = xhps[c].tile([128, 512], F32, tag="xhp")
            g = (p % 2) * 64
            colbase = (p // 2) * 512 + c * 256
            xrhs = xsb[g : g + 64, colbase : colbase + 256]
            for blk in (0, 1):
                nc.tensor.matmul(
                    xhp[:, blk * 256 : (blk + 1) * 256],
                    wi[g : g + 64, (4 + blk) * 128 : (5 + blk) * 128],
                    xrhs, start=(blk == 0), stop=(blk == 1),
                    tile_position=(g, 0))
            xh_sb = xhpool[c].tile([128, 512], F16)
            return xhp, xh_sb

        def xh_copy_half(xhp, xh_sb, tu):
            src = xhp[:].rearrange("p (k b) -> p k b", k=2, b=256)
            dst = xh_sb[:].rearrange("p (k b) -> p k b", k=2, b=256)
            nc.scalar.activation(dst[:, :, tu * 128 : tu * 128 + 128],
                                 src[:, :, tu * 128 : tu * 128 + 128],
                                 AF.Copy)

        def step(t, c, rt, zt, rht, xh_sb):
            tu = t % 2
            hc = h[c]

            def rec_mm1(m, ct, out, stop):
                lhsT = wr[:, (ct * 6 + m) * 128 : (ct * 6 + m + 1) * 128]
                rhs = hc[:, ct * 128 : (ct + 1) * 128]
                nc.tensor.matmul(out, lhsT, rhs, start=False, stop=stop)

            def rec_mm(m, out, stop):
                rec_mm1(m, 0, out, False)
                rec_mm1(m, 1, out, stop)

            r_sb = gpool[c].tile([128, 256], F16, tag="r")
            z_sb = gpool[c].tile([128, 256], F16, tag="z")

            # rec r (closes R bank -> sig-r fires after 4 MMs)
            rec_mm(2, rt[:, 0:128], False)
            rec_mm(3, rt[:, 128:256], True)
            nc.scalar.activation(r_sb[:], rt[:, 0:256], AF.Sigmoid)
            # rec rh (closes RH bank)
            rec_mm(4, rht[:, 0:128], False)
            rec_mm(5, rht[:, 128:256], True)
            # rec z (closes Z bank), sig-z
            rec_mm(0, zt[:, 0:128], False)
            rec_mm(1, zt[:, 128:256], True)
            nc.scalar.activation(z_sb[:], zt[:, 0:256], AF.Sigmoid)

            xh_in = xh_sb[:].rearrange("p (k b) -> p k b", k=2, b=256)
            xh_in = xh_in[:, :, tu * 128 : tu * 128 + 128]

            # off-chain op on GPSIMD: u = z * h
            zc = gpool[c].tile([128, 256], F16, tag="zc")
            u_t = gpool[c].tile([128, 256], F16, tag="u")
            nc.gpsimd.tensor_tensor(u_t[:], z_sb[:], hc[:], ALU.mult)

            p_t = gpool[c].tile([128, 256], F16, tag="p")
            pre = gpool[c].tile([128, 256], F16, tag="pre")
            w_t = gpool[c].tile([128, 256], F16, tag="w")
            nc.vector.tensor_tensor(p_t[:], rht[:], r_sb[:], ALU.mult)
            pre_out = pre[:].rearrange("p (k b) -> p k b", k=2, b=128)
            p_in = p_t[:].rearrange("p (k b) -> p k b", k=2, b=128)
            nc.vector.tensor_tensor(pre_out, p_in, xh_in, ALU.add)
            # zc emitted late so its sig-z dep never clogs the queues
            nc.vector.tensor_scalar(zc[:], z_sb[:], -1.0, 1.0,
                                    op0=ALU.mult, op1=ALU.add)
            # w = relu(pre) * (1-z)
            nc.vector.scalar_tensor_tensor(
                w_t[:], pre[:], 0.0, zc[:], op0=ALU.max, op1=ALU.mult)
            if skew_src[0] is None:
                skew_src[0] = p_t
            hn = hpool[c].tile([128, 256], F16)
            nc.vector.tensor_tensor(hn[:], u_t[:], w_t[:], ALU.add)
            h[c] = hn

        # priming: x tiles for t=0, xh pair 0 + its first-half copy
        cur = [x_step(0, 0), x_step(0, 1)]
        xpair_cur = [x_pair(0, 0), x_pair(0, 1)]
        for c in (0, 1):
            xh_copy_half(*xpair_cur[c], 0)
        for t in range(T):
            for c in (0, 1):
                if t == 0 and c == 1:
                    h0b = hpool[1].tile([128, 256], F16)
                    nc.vector.memset(h0b[:], 0.0)
                    h[1] = h0b
                rt, zt, rht = cur[c]
                step(t, c, rt, zt, rht, xpair_cur[c][1])
                if t + 1 < T:
                    cur[c] = x_step(t + 1, c)
            if t % 2 == 0 and t + 1 < T:
                # second-half copy of the current pair (needed at t+1)
                for c in (0, 1):
                    xh_copy_half(*xpair_cur[c], 1)
            elif t % 2 == 1 and t + 2 < T:
                # next pair's MMs + first-half copy (needed at t+2)
                for c in (0, 1):
                    xpair_cur[c] = x_pair((t + 1) // 2, c)
                    xh_copy_half(*xpair_cur[c], 0)

        # ---- BN + dense epilogue: y = s . h + c ----
        fin = zpool[0].tile([128, 256], F32, tag="z")
        first = True
        for c in (0, 1):
            for ct in (0, 1):
                nc.tensor.matmul(
                    fin[0:1, c * 128 : (c + 1) * 128],
                    sv[:, ct : ct + 1],
                    h[c][:, ct * 128 : (ct + 1) * 128],
                    start=first,
                    stop=(c == 1 and ct == 1),
                )
                first = False
        ysb = const.tile([1, 256], F32)
        nc.vector.tensor_scalar_add(ysb[:], fin[0:1, 0:256], cv[0:1, 0:1])
        nc.sync.dma_start(y_d.ap(), ysb[:])

    nc.compile()
    return nc


BN_EPS = 1e-3


def prep_core_inputs(x_core, kernel, rec_kernel, bias, gamma, beta,
                     moving_mean, moving_var, dense_w, dense_b):
    """Host-side prep of one core's input dict. x_core: [B=256, T, 32] f32."""
    import numpy as np
    B, T, F = x_core.shape
    npair = T // 2
    xp = np.zeros((128, (npair + 1) // 2 * 512), np.float16)
    xt = np.ascontiguousarray(x_core.astype(np.float16).transpose(1, 2, 0))  # [T,F,B]
    for p in range(npair):
        g, colbase = p % 2, (p // 2) * 512
        for c in (0, 1):
            for tu in (0, 1):
                blk = xt[2 * p + tu][:, c * 128 : (c + 1) * 128]  # [F,128]
                cols = slice(colbase + c * 256 + tu * 128,
                             colbase + c * 256 + tu * 128 + 128)
                xp[64 * g : 64 * g + 32, cols] = blk
                xp[64 * g + 32, cols] = 1.0

    rec = rec_kernel.astype(np.float16)
    wr = np.concatenate(
        [rec[ct * 128 : (ct + 1) * 128, m * 128 : (m + 1) * 128]
         for ct in (0, 1) for m in range(6)],
        axis=1,
    )
    b_comb = np.concatenate([
        bias[0, 0:256] + bias[1, 0:256],
        bias[0, 256:512] + bias[1, 256:512],
        bias[0, 512:768],
    ])
    wi = np.zeros((128, 768), np.float16)
    for g in (0, 1):
        wi[64 * g : 64 * g + 32, :] = kernel.astype(np.float16)
        wi[64 * g + 32, :] = b_comb.astype(np.float16)
    brh = bias[1, 512:768].astype(np.float16).reshape(1, 256)

    rs = 1.0 / np.sqrt(moving_var + BN_EPS)
    s = (gamma * rs * dense_w[:, 0]).astype(np.float16)
    sv = np.stack([s[:128], s[128:]], axis=1)
    cc = dense_b[0] + np.sum((beta - moving_mean * gamma * rs) * dense_w[:, 0])
    cv = np.array([[cc]], np.float32)
    return {
        "xp": np.ascontiguousarray(xp),
        "wr": np.ascontiguousarray(wr),
        "wi": wi,
        "brh": brh,
        "sv": np.ascontiguousarray(sv),
        "cv": cv,
    }

_NC_CACHE = {}


def _get_nc():
    if "nc" not in _NC_CACHE:
        _NC_CACHE["nc"] = build_gru_nc(T=256)
    return _NC_CACHE["nc"]


def kernel(x, kernel, rec_kernel, bias, gamma, beta, moving_mean, moving_var,
           dense_w, dense_b):
    import numpy as np
    from concourse.bass_utils import run_bass_kernel_spmd

    x = np.asarray(x, dtype=np.float32)
    args = [np.asarray(a, dtype=np.float32) for a in
            (kernel, rec_kernel, bias, gamma, beta, moving_mean, moving_var,
             dense_w, dense_b)]
    nc = _get_nc()
    n_cores = 8
    nb = x.shape[0] // n_cores
    in_maps = [prep_core_inputs(x[i * nb : (i + 1) * nb], *args)
               for i in range(n_cores)]
    res = run_bass_kernel_spmd(nc, in_maps, core_ids=list(range(n_cores)))
    return np.concatenate(
        [res.results[i]["y"].reshape(nb, 1) for i in range(n_cores)], axis=0
    ).astype(np.float32)
